# revision 51
# baseline (speedup 1.0000x reference)
"""Trainium2 Bass kernel for a decoder block (MHA + GELU MLP, pre-LN, causal).

Problem shapes (hardcoded): B=2, T=2048, C=512, H=8, HD=64, f32 in/out.

Sharding: 8 cores = 2 batches x 4 query-blocks of 512 tokens. Every core
receives its batch's full x *rotated* so that its query block sits at token
rows 1536:2048 — the SPMD program is identical across cores while the causal
structure moves into per-core input data:
  - a per-slot exp bias (0 or -1e30) kills fully-masked 128-token key tiles,
  - a static lower-triangular mask (added to scores via an identity matmul)
    handles the diagonal 512x512 block, identical for every core.

All matmul operands are bf16 (weights cast host-side, activations cast at
eviction): the fp32r self-loading weight path runs the PE at a fraction of
peak, while bf16 streams one row/cycle with a separately schedulable weight
load. PSUM accumulation stays fp32, the residual spine (x, x2, output) stays
fp32, so the extra rounding lands only on matmul operands.

On-chip layout avoids all transposes except h = ln(x) -> hT (PE transposes):
scores are computed as S^T[k, q] (keys on partitions) so the softmax
denominator comes from a ones-column appended to V, probability tiles feed
the PV matmul directly as lhsT, and attention output appears as attnT[d, q],
which is exactly the lhsT the output projection needs. The two head-halves of
a key tile share their exp bias, so both score tiles land in one 2-bank
[128,1024] PSUM tile and a single ACT exp covers them. LN gains/biases are
folded into adjacent weights host-side; bo/b1/b2 are applied via ones-row
rank-1 matmuls or the ACT bias operand.

Engine balance: LN stats stay on DVE; evictions are spread DVE/ACT; softmax
denominators take a DMA round-trip that reshapes the [1,512] row to [128,4]
so the reciprocal uses every DVE lane (the 1-partition InstReciprocal is
~3us). The Q^T tiles are zero-padded per head to a full 128-row contraction:
K=64 score matmuls leave the PE clock gate throttled at 1.2 GHz for the whole
attention phase, while full-row matmuls keep it at 2.4 GHz. Scores and exp
run one step ahead of the PV matmuls so the PE never waits on the activation
engine. All of x is DMA'd before any weights so phase 1 is never
input-starved.
"""

import os
import sys

for _p in ("/opt/trn_rl_repo",):
    if _p not in sys.path and os.path.isdir(_p):
        sys.path.insert(0, _p)

import ml_dtypes
import numpy as np

import concourse.bacc as bacc
import concourse.bass as bass
import concourse.tile as tile
from concourse import mybir
from concourse.bass_utils import run_bass_kernel_spmd

F32 = mybir.dt.float32
B16 = mybir.dt.bfloat16
AF = mybir.ActivationFunctionType
BF = ml_dtypes.bfloat16

B, T, C, H, HD = 2, 2048, 512, 8, 64
NCORES = 8
QB = 512          # query-block tokens per core
NT = T // 128     # 16 key tiles
NQ = QB // 128    # 4 query tiles per core
NEG = -1.0e30

last_run = None       # test harness reads exec_time_ns from here
_prog_cache = {}


def _build_program(with_qkv_bias):
    nc = bacc.Bacc("TRN2", target_bir_lowering=False, debug=False,
                   num_devices=NCORES)

    xb_d = nc.dram_tensor("xb", [T, C], F32, kind="ExternalInput")
    wq_d = nc.dram_tensor("wq", [128, 4, 512], B16, kind="ExternalInput")
    wk_d = nc.dram_tensor("wk", [128, 4, 512], B16, kind="ExternalInput")
    wv_d = nc.dram_tensor("wv", [128, 4, 512], B16, kind="ExternalInput")
    wo_d = nc.dram_tensor("wo", [128, 4, 512], B16, kind="ExternalInput")
    w1_d = nc.dram_tensor("w1", [16, 128, 512], B16, kind="ExternalInput")
    w2_d = nc.dram_tensor("w2", [16, 128, 512], B16, kind="ExternalInput")
    kb_d = nc.dram_tensor("kbias", [128, 16], F32, kind="ExternalInput")
    bo_d = nc.dram_tensor("bo", [1, 512], B16, kind="ExternalInput")
    b1_d = nc.dram_tensor("b1c", [128, 16], F32, kind="ExternalInput")
    b2_d = nc.dram_tensor("b2r", [1, 512], B16, kind="ExternalInput")
    id_d = nc.dram_tensor("identc", [128, 128], B16, kind="ExternalInput")
    mk_d = nc.dram_tensor("maskc", [128, 4, 512], B16, kind="ExternalInput")
    on_d = nc.dram_tensor("onesc", [128, 512], B16, kind="ExternalInput")
    bq_d = (nc.dram_tensor("bqkv", [3, 1, 512], B16, kind="ExternalInput")
            if with_qkv_bias else None)
    xqb_d = nc.dram_tensor("xqb", [QB, C], F32, kind="ExternalInput")
    out_d = nc.dram_tensor("out", [QB, C], F32, kind="ExternalOutput")

    with tile.TileContext(nc) as tc:
        with (
            tc.tile_pool(name="const", bufs=1) as const,
            tc.tile_pool(name="mid", bufs=1) as mid,
            tc.tile_pool(name="tp", bufs=3) as tp,
            tc.tile_pool(name="sp", bufs=4) as sp,
        ):
            # ---- x first: 16 tile DMAs fan out across all queues ----
            xt_sb = const.tile([128, 16, 512], F32)
            for t in range(NT):
                nc.sync.dma_start(xt_sb[:, t, :], xb_d[bass.ts(t, 128), :])

            # ---- small consts needed by phase 1 ----
            ident = const.tile([128, 128], B16)
            nc.sync.dma_start(ident[:], id_d[:])
            eps_sb = const.tile([128, 1], F32)
            nc.vector.memset(eps_sb[:], 1e-5)
            kb_sb = const.tile([128, 16], F32)
            nc.sync.dma_start(kb_sb[:], kb_d[:])
            ones512 = const.tile([128, 512], B16)
            nc.sync.dma_start(ones512[:], on_d[:])
            ones_sb = ones512  # [1, 128] slices come from row 0

            # ---- QKV weights (needed ~20us in) ----
            wq_sb = const.tile([128, 4, 512], B16)
            nc.sync.dma_start(wq_sb[:], wq_d[:])
            wk_sb = const.tile([128, 4, 512], B16)
            nc.sync.dma_start(wk_sb[:], wk_d[:])
            wv_sb = const.tile([128, 4, 512], B16)
            nc.sync.dma_start(wv_sb[:], wv_d[:])
            if with_qkv_bias:
                bq_sb = const.tile([3, 1, 512], B16)
                nc.sync.dma_start(bq_sb[:], bq_d[:])

            # ---- attention consts ----
            mask_sb = const.tile([128, 4, 512], B16)
            nc.sync.dma_start(mask_sb[:], mk_d[:])
            wo_sb = const.tile([128, 4, 512], B16)
            nc.sync.dma_start(wo_sb[:], wo_d[:])
            # x[qblock] + bo, precomputed host-side: the out-projection's
            # residual add picks up the bias for free (no K=1 bias matmul)
            xqb_sb = const.tile([128, 4, 512], F32)
            for t in range(NQ):
                nc.sync.dma_start(xqb_sb[:, t, :], xqb_d[bass.ts(t, 128), :])

            # ---- FFN weights (needed last; DMA'd during earlier phases) ----
            w1_sb = const.tile([128, 16, 512], B16)
            for f in range(16):
                nc.sync.dma_start(w1_sb[:, f, :], w1_d[f])
            w2_sb = const.tile([128, 16, 512], B16)
            for f in range(16):
                nc.sync.dma_start(w2_sb[:, f, :], w2_d[f])
            b1_sb = const.tile([128, 16], F32)
            nc.sync.dma_start(b1_sb[:], b1_d[:])
            b2_sb = const.tile([1, 512], B16)
            nc.sync.dma_start(b2_sb[:], b2_d[:])

            # ---------------- persistent mid tensors ----------------
            kt_sb = mid.tile([128, 4, 2048], B16)   # K^T  (head pair, 64h+d)
            v_sb = mid.tile([128, 16, 520], B16)    # V + ones column per head
            # Q^T per head, padded to K=128: head h occupies rows 0:64 (even
            # h) or 64:128 (odd h) matching its position in the kt pair
            # layout; the other 64 rows stay zero so a full-128-row score
            # matmul contracts to exactly that head's scores. Full-row
            # matmuls keep the PE's activity monitor busy (K=64 streams left
            # the clock gate at 4/8 for the whole attention phase).
            qt_sb = mid.tile([128, 8, 512], B16)
            nc.vector.memset(qt_sb[:], 0.0)
            # pre-set the ones columns (col 64 of each 65-wide head group)
            vones = (v_sb[:, :, :]
                     .rearrange("p a (h e) -> p a h e", e=65)[:, :, :, 64:65])
            nc.vector.tensor_copy(
                vones, ones512[:, 0:128]
                .rearrange("p (a h) -> p a h", h=8).unsqueeze(3))

            def layernorm_to(src_ap, dst_ap, apply_eng=None):
                st = sp.tile([128, 6], F32, tag="st")
                nc.vector.bn_stats(out=st[:], in_=src_ap)
                mv = sp.tile([128, 2], F32, tag="mv")
                nc.vector.bn_aggr(out=mv[:], in_=st[:])
                lg = sp.tile([128, 1], F32, tag="lg")
                nc.scalar.activation(out=lg[:], in_=mv[:, 1:2], func=AF.Sqrt,
                                     bias=eps_sb[:])
                rs = sp.tile([128, 1], F32, tag="rs")
                nc.vector.reciprocal(out=rs[:], in_=lg[:])
                (apply_eng or nc.vector).tensor_scalar(
                    out=dst_ap, in0=src_ap, scalar1=mv[:, 0:1], scalar2=rs[:],
                    op0=mybir.AluOpType.subtract, op1=mybir.AluOpType.mult)

            # ======== phase 1+2 scope: LN1, transpose, Q/K/V ========
            with tc.tile_pool(name="p1", bufs=1) as p1:
                h1t_sb = p1.tile([128, 4, 2048], B16)

                with tc.tile_pool(name="ptr1", bufs=2, space="PSUM") as ptr1:
                    for t in range(NT):
                        ht = tp.tile([128, 512], B16, tag="ht")
                        layernorm_to(xt_sb[:, t, :], ht[:])
                        pst = ptr1.tile([128, 4, 128], B16, tag="tr")
                        for cc in range(4):
                            nc.tensor.transpose(
                                pst[:, cc, :], ht[:, bass.ts(cc, 128)],
                                ident[:])
                        ev = h1t_sb[:, :, bass.ts(t, 128)]
                        # evictions on ACT (Copy shares every table; DVE owns
                        # the LN stats, so route these off it)
                        nc.scalar.copy(ev, pst[:])

                # throwaway exp: pulls the sqrt->exp ACT table switch
                # (~1.3us) off the first real attention exp, which sits on
                # the ACT critical path. All phase-2 ACT ops are Copies,
                # present in every table.
                atl_sb = p1.tile([128, 1], F32)
                nc.scalar.activation(out=atl_sb[:], in_=eps_sb[:],
                                     func=AF.Exp)

                with tc.tile_pool(name="pq", bufs=2, space="PSUM") as pq_ps:
                    # Q^T: head pairs; rhs = h1T of the query block
                    for pr in range(4):
                        ps = pq_ps.tile([128, 512], F32, tag="ps")
                        for cc in range(4):
                            nc.tensor.matmul(
                                ps[:], wq_sb[:, cc, bass.ts(pr, 128)],
                                h1t_sb[:, cc, 1536:2048],
                                start=(cc == 0),
                                stop=(cc == 3 and not with_qkv_bias))
                        if with_qkv_bias:
                            nc.tensor.matmul(
                                ps[:], bq_sb[0, :, bass.ts(pr, 128)],
                                ones512[:], start=False, stop=True)
                        nc.vector.tensor_copy(qt_sb[0:64, 2 * pr, :],
                                              ps[0:64, :])
                        nc.vector.tensor_copy(qt_sb[64:128, 2 * pr + 1, :],
                                              ps[64:128, :])

                    # K^T: head pairs x 4 key chunks of 512; cc outer so the
                    # stationary wk chunk is loaded once per 4 matmuls
                    with tc.tile_pool(name="pk", bufs=1, space="PSUM") as pk_ps:
                        for pr in range(4):
                            psk = [pk_ps.tile([128, 512], F32, tag=f"k{nk}",
                                              name=f"k{nk}")
                                   for nk in range(4)]
                            for cc in range(4):
                                for nk in range(4):
                                    nc.tensor.matmul(
                                        psk[nk][:],
                                        wk_sb[:, cc, bass.ts(pr, 128)],
                                        h1t_sb[:, cc, bass.ts(nk, 512)],
                                        start=(cc == 0),
                                        stop=(cc == 3 and not with_qkv_bias),
                                        skip_group_check=True)
                            for nk in range(4):
                                if with_qkv_bias:
                                    nc.tensor.matmul(
                                        psk[nk][:],
                                        bq_sb[1, :, bass.ts(pr, 128)],
                                        ones512[:], start=False, stop=True,
                                        skip_group_check=True)
                                ev = kt_sb[:, pr, bass.ts(nk, 512)]
                                if nk % 2 == 0:
                                    nc.vector.tensor_copy(ev, psk[nk][:])
                                else:
                                    nc.scalar.copy(ev, psk[nk][:])

                    # V: 16 token tiles; rhs = all heads of Wv at once
                    for t in range(NT):
                        ps = pq_ps.tile([128, 512], F32, tag="ps")
                        for cc in range(4):
                            nc.tensor.matmul(
                                ps[:], h1t_sb[:, cc, bass.ts(t, 128)],
                                wv_sb[:, cc, :],
                                start=(cc == 0),
                                stop=(cc == 3 and not with_qkv_bias))
                        if with_qkv_bias:
                            nc.tensor.matmul(
                                ps[:], ones_sb[0:1, 0:128], bq_sb[2],
                                start=False, stop=True)
                        ev = (v_sb[:, t, :]
                              .rearrange("p (h e) -> p h e", e=65)[:, :, 0:64])
                        sv = ps[:].rearrange("p (h e) -> p h e", e=64)
                        if t % 2 == 0:
                            nc.scalar.copy(ev, sv)
                        else:
                            nc.vector.tensor_copy(ev, sv)

            # ======== phases 3..7 scope ========
            with tc.tile_pool(name="mid2", bufs=1) as mid2:
                at_sb = mid2.tile([128, 4, 512], B16)    # attnT (normalized)
                x2_sb = mid2.tile([128, 4, 512], F32)    # post-attn residual
                h2t_sb = mid2.tile([128, 4, 512], B16)   # ln2(x2)^T
                g_sb = mid2.tile([128, 16, 512], B16)    # gelu(ffn1)^T

                # -------- phase 3: attention --------
                def attention_pr(pr, po, ps_ps, ap_pool, drp):
                    def pv(s, ptile):
                        for half in range(2):
                            h = 2 * pr + half
                            nc.tensor.matmul(
                                po[half][:],
                                v_sb[:, s, h * 65:(h + 1) * 65],
                                ptile[:, bass.ts(half, 512)],
                                start=(s == 0), stop=(s == NT - 1),
                                skip_group_check=True)

                    # scores+exp run one step ahead of the PV matmuls so
                    # the PE never sits at the head of a PV waiting for
                    # the exp of the same step.
                    prev = None
                    for s in range(NT):
                        ptile = ap_pool.tile([128, 1024], B16, tag="pt")
                        pss = ps_ps.tile([128, 1024], F32, tag="ps")
                        for half in range(2):
                            dst = pss[:, bass.ts(half, 512)]
                            nc.tensor.matmul(
                                dst,
                                kt_sb[:, pr, bass.ts(s, 128)],
                                qt_sb[:, 2 * pr + half, :],
                                start=True, stop=(s < 12))
                            if s >= 12:
                                nc.tensor.matmul(
                                    dst, ident[:],
                                    mask_sb[:, s - 12, :],
                                    start=False, stop=True)
                        nc.scalar.activation(
                            out=ptile[:], in_=pss[:], func=AF.Exp,
                            bias=kb_sb[:, s:s + 1])
                        if prev is not None:
                            pv(*prev)
                        prev = (s, ptile)
                    pv(*prev)
                    for half in range(2):
                        base = 64 * half
                        # denom row -> DRAM -> [128,4] so the reciprocal
                        # uses all DVE lanes, then back out as a 64-row
                        # broadcast for the normalize multiply.
                        dn = ap_pool.tile([1, 512], F32, tag="dn")
                        nc.vector.tensor_copy(dn[:], po[half][64:65, :])
                        rd = drp.tile([1, 512], F32, tag="rd")
                        nc.sync.dma_start(rd[:], dn[:])
                        rc = ap_pool.tile([128, 4], F32, tag="rc")
                        nc.sync.dma_start(
                            rc[:], rd[:].rearrange("o (p f) -> (o p) f",
                                                   p=128))
                        rr = ap_pool.tile([128, 4], F32, tag="rr")
                        nc.vector.reciprocal(out=rr[:], in_=rc[:])
                        rd2 = drp.tile([128, 4], F32, tag="rd2")
                        nc.sync.dma_start(rd2[:], rr[:])
                        rb = ap_pool.tile([64, 512], F32, tag="rb")
                        nc.sync.dma_start(
                            rb[:],
                            rd2[:].rearrange("(o p) f -> o (p f)", o=1)
                            .to_broadcast([64, 512]))
                        nc.vector.tensor_mul(
                            out=at_sb[base:base + 64, pr, :],
                            in0=po[half][0:64, :], in1=rb[:])

                with (
                    tc.tile_pool(name="psS", bufs=2, space="PSUM") as ps_ps,
                    tc.tile_pool(name="psO", bufs=2, space="PSUM") as po_ps,
                    tc.tile_pool(name="ap", bufs=4) as ap_pool,
                    tc.tile_pool(name="drp", bufs=2, space="DRAM") as drp,
                ):
                    for pr in range(4):
                        po = [po_ps.tile([65, 512], F32, tag=f"po{i}",
                                         name=f"po{i}")
                              for i in range(2)]
                        attention_pr(pr, po, ps_ps, ap_pool, drp)

                    # ~4us of throwaway matmuls inside this pool scope: the
                    # PE would otherwise idle >3.4us on the last head-pair's
                    # denominator round-trip (plus the pool-close barrier),
                    # dropping the clock gate to 4/8 and running the whole
                    # FFN at 1.2 GHz. Emitted after the pr3 normalize ops so
                    # they fill exactly that wait.
                    wt = ps_ps.tile([128, 1024], F32, tag="ps")
                    for _ in range(20):
                        nc.tensor.matmul(wt[:, 0:512], ident[:],
                                         mask_sb[:, 0, :], start=True,
                                         stop=True, skip_group_check=True)

                with tc.tile_pool(name="pf", bufs=4, space="PSUM") as pf_ps:
                    # -------- phase 4: output projection + residual --------
                    for qt in range(NQ):
                        ps = pf_ps.tile([128, 512], F32, tag="pf")
                        for cc in range(4):
                            nc.tensor.matmul(
                                ps[:], at_sb[:, cc, bass.ts(qt, 128)],
                                wo_sb[:, cc, :], start=(cc == 0),
                                stop=(cc == 3))
                        nc.vector.tensor_add(out=x2_sb[:, qt, :], in0=ps[:],
                                             in1=xqb_sb[:, qt, :])

                    # -------- phase 5: LN2 + transpose --------
                    with tc.tile_pool(name="ptr2", bufs=2,
                                      space="PSUM") as ptr2:
                        for qt in range(NQ):
                            ht = tp.tile([128, 512], B16, tag="ht")
                            layernorm_to(x2_sb[:, qt, :], ht[:])
                            pst = ptr2.tile([128, 4, 128], B16, tag="tr")
                            for cc in range(4):
                                nc.tensor.transpose(
                                    pst[:, cc, :],
                                    ht[:, bass.ts(cc, 128)], ident[:])
                            ev = h2t_sb[:, :, bass.ts(qt, 128)]
                            nc.scalar.copy(ev, pst[:])

                    # -------- phase 6: FFN1 + gelu --------
                    for f in range(16):
                        ps = pf_ps.tile([128, 512], F32, tag="pf")
                        for cc in range(4):
                            nc.tensor.matmul(
                                ps[:], w1_sb[:, f, bass.ts(cc, 128)],
                                h2t_sb[:, cc, :],
                                start=(cc == 0), stop=(cc == 3))
                        nc.scalar.activation(
                            out=g_sb[:, f, :], in_=ps[:], func=AF.Gelu,
                            bias=b1_sb[:, f:f + 1])

                    # -------- phase 7: FFN2 + residual + store --------
                    with (
                        tc.tile_pool(name="pf2", bufs=1,
                                     space="PSUM") as pf2_ps,
                        tc.tile_pool(name="op", bufs=2) as op,
                    ):
                        pso = [pf2_ps.tile([128, 512], F32, tag=f"o{qt}",
                                           name=f"o{qt}") for qt in range(NQ)]
                        # bias seeds the accumulators up front so only the
                        # residual add + store trail the last weight matmul
                        for qt in range(NQ):
                            nc.tensor.matmul(
                                pso[qt][:], ones_sb[0:1, 0:128], b2_sb[:],
                                start=True, stop=False, skip_group_check=True)
                        for ff in range(16):
                            for qt in range(NQ):
                                nc.tensor.matmul(
                                    pso[qt][:],
                                    g_sb[:, ff, bass.ts(qt, 128)],
                                    w2_sb[:, ff, :], start=False,
                                    stop=(ff == 15), skip_group_check=True)
                                if ff == 15:
                                    ot = op.tile([128, 512], F32, tag="ot")
                                    nc.vector.tensor_add(
                                        out=ot[:], in0=pso[qt][:],
                                        in1=x2_sb[:, qt, :])
                                    nc.sync.dma_start(
                                        out_d[bass.ts(qt, 128), :], ot[:])

    nc.compile()
    return nc


def _host_prep(x, Wq, Wk, Wv, Wo, bo, W1, b1, W2, b2, g1, be1, g2, be2):
    """Fold LN gains into weights; build per-core rotated inputs/slot biases."""
    x = np.asarray(x, np.float32)
    g1 = np.asarray(g1, np.float32)
    be1 = np.asarray(be1, np.float32)
    g2 = np.asarray(g2, np.float32)
    be2 = np.asarray(be2, np.float32)

    wq_cat = np.transpose(np.asarray(Wq, np.float32), (1, 0, 2)).reshape(C, H * HD)
    wk_cat = np.transpose(np.asarray(Wk, np.float32), (1, 0, 2)).reshape(C, H * HD)
    wv_cat = np.transpose(np.asarray(Wv, np.float32), (1, 0, 2)).reshape(C, H * HD)
    scl = float(HD) ** -0.5
    wq_f = (g1[:, None] * wq_cat) * scl
    wk_f = g1[:, None] * wk_cat
    wv_f = g1[:, None] * wv_cat
    bq = (be1 @ wq_cat) * scl
    bk = be1 @ wk_cat
    bv = be1 @ wv_cat
    with_qkv_bias = bool(np.any(bq) or np.any(bk) or np.any(bv))

    W1 = np.asarray(W1, np.float32)
    w1_f = g2[:, None] * W1
    b1_f = np.asarray(b1, np.float32) + be2 @ W1

    maskc = np.zeros((4, 128, 512), np.float32)
    qidx = np.arange(512)[None, :]
    for dd in range(4):
        pidx = 128 * dd + np.arange(128)[:, None]
        maskc[dd] = np.where(pidx <= qidx, 0.0, NEG)

    common = {
        "identc": np.eye(128, dtype=BF),
        "maskc": np.ascontiguousarray(maskc.transpose(1, 0, 2)).astype(BF),
        "onesc": np.ones((128, 512), BF),
        "wq": np.ascontiguousarray(
            wq_f.reshape(4, 128, 512).transpose(1, 0, 2)).astype(BF),
        "wk": np.ascontiguousarray(
            wk_f.reshape(4, 128, 512).transpose(1, 0, 2)).astype(BF),
        "wv": np.ascontiguousarray(
            wv_f.reshape(4, 128, 512).transpose(1, 0, 2)).astype(BF),
        "wo": np.ascontiguousarray(
            np.asarray(Wo, np.float32).reshape(4, 128, 512)
            .transpose(1, 0, 2)).astype(BF),
        "w1": np.ascontiguousarray(
            w1_f.reshape(4, 128, 16, 128).transpose(2, 1, 0, 3)
            .reshape(16, 128, 512)).astype(BF),
        "w2": np.ascontiguousarray(
            np.asarray(W2, np.float32).reshape(16, 128, 512)).astype(BF),
        "bo": np.asarray(bo, np.float32).reshape(1, 512).astype(BF),
        "b1c": np.ascontiguousarray(b1_f.reshape(16, 128).T),
        "b2r": np.asarray(b2, np.float32).reshape(1, 512).astype(BF),
    }
    if with_qkv_bias:
        common["bqkv"] = np.ascontiguousarray(
            np.stack([bq, bk, bv]).reshape(3, 1, 512)).astype(BF)

    in_maps = []
    for c in range(NCORES):
        bb, j = c // 4, c % 4
        o = QB * j
        xb_rot = np.roll(x[bb], 1536 - o, axis=0)
        kbias = np.zeros(16, np.float32)
        for s in range(12):
            m = (s - 12 + 4 * j) % 16
            if m >= 4 * j:       # original key tile at/after the q block
                kbias[s] = NEG
        im = dict(common)
        im["xb"] = np.ascontiguousarray(xb_rot)
        im["xqb"] = np.ascontiguousarray(
            xb_rot[1536:2048] + np.asarray(bo, np.float32)[None, :])
        im["kbias"] = np.ascontiguousarray(
            np.broadcast_to(kbias.reshape(1, 16), (128, 16)))
        in_maps.append(im)
    return in_maps, with_qkv_bias


def kernel(**inputs):
    global last_run
    in_maps, with_qkv_bias = _host_prep(**inputs)
    if with_qkv_bias not in _prog_cache:
        _prog_cache[with_qkv_bias] = _build_program(with_qkv_bias)
    nc = _prog_cache[with_qkv_bias]
    res = run_bass_kernel_spmd(nc, in_maps, list(range(NCORES)))
    last_run = res
    out = np.empty((B, T, C), np.float32)
    for c in range(NCORES):
        bb, j = c // 4, c % 4
        out[bb, QB * j:QB * (j + 1), :] = res.results[c]["out"]
    return out
